# revision 34
# baseline (speedup 1.0000x reference)
"""Trainium2 Bass kernel for nn_DNBNSystem (gnn_message_passing).

Sharding: expert-parallel — one graph node per NeuronCore (N=8 nodes, 8 cores).
Each core runs the conv feature extractor + recurrent controller/attention
update for its node over the full batch B=256. The inter-node attention
exchanges (k, v*send) per step via per-batch-chunk AllGather in bf16.

All matmuls run in bf16 (fp32 PSUM accumulate). Conv1 output is stored in an
x-parity-split layout so conv2's strided taps become packed bf16 reads.

Self-contained: hardcodes shapes; builds the Bass program once and caches it.
"""
import os
import numpy as np
import ml_dtypes

import bass_rust
import concourse.bass as bass
import concourse.mybir as mybir
import concourse.tile as tile
from concourse.vector_clock import ScopedClock
from concourse.masks import make_identity
from concourse.bass_utils import run_bass_kernel_spmd

dt = mybir.dt
AF = mybir.ActivationFunctionType
ALU = mybir.AluOpType
AX = mybir.AxisListType

# ----- problem constants -----
N, B, M, C, NH, S_, HC, T, OUT = 8, 256, 512, 512, 8, 8, 64, 3, 100
DH = C // NH          # 64
P = 128
NBCH = B // P         # 2 batch chunks of 128
KM = M // P           # 4 feature chunks
NCORE = 8
GB = 64               # conv batch-group size
NG = B // GB          # 4 conv groups

TRACE = False
_CACHE = {}


# ---------------------------------------------------------------------------
# Walrus workarounds: this build accepts only ONE sync wait per instruction.
# ---------------------------------------------------------------------------
def _patched_drain_and_barrier(self, tick_clock, wait_clock):
    nc = self.nc
    drain_inst = nc.sync.drain()
    wait_clock.add_sem_waits(
        drain_inst.ins, ScopedClock({None: tick_clock.global_clock})
    )
    si = drain_inst.ins.sync_info
    waits = list(si.on_wait)
    if len(waits) > 1:
        drain_inst.ins.sync_info = bass_rust.SyncInfo(
            on_wait=waits[:1], on_update=list(si.on_update)
        )
        handles = {h.name: h for h in self.sems.allocated().values()}
        for w in waits[1:]:
            d2 = nc.sync.drain()
            d2.wait_op(handles[w.ant_name], w.wait_value, "sem-ge")
    nc.all_engine_barrier()
    popped = nc._tile_sem_poison_stack.pop()
    assert popped is self._sem_poison
    nc.clear_and_free_semaphores(list(self.sems.allocated().values()))
    nc.all_engine_barrier()


tile.TileContext._drain_and_barrier = _patched_drain_and_barrier


def _split_multiwaits(nc, max_waits=1):
    counter = 0
    for fn in nc.m.functions:
        for bb in fn.blocks:
            lst = bb.instructions
            i = 0
            while i < len(lst):
                inst = lst[i]
                si = inst.sync_info
                if si is not None and len(si.on_wait) > max_waits:
                    waits = list(si.on_wait)
                    sem_waits = [w for w in waits if w.sync_type == "semaphore"]
                    other = [w for w in waits if w.sync_type != "semaphore"]
                    n_keep = max(1, max_waits - len(other))
                    keep, hoist = sem_waits[-n_keep:], sem_waits[:-n_keep]
                    for w in hoist:
                        nop = mybir.InstNoOp(name=f"WSPLIT-{counter}")
                        counter += 1
                        nop.engine = inst.engine
                        nop.sync_info = bass_rust.SyncInfo(on_wait=[w], on_update=[])
                        lst.insert(i, nop)
                        i += 1
                    inst.sync_info = bass_rust.SyncInfo(
                        on_wait=other + keep, on_update=list(si.on_update)
                    )
                i += 1


# ---------------------------------------------------------------------------
# Program builder (SPMD: all cores run this program on their node's weights).
# ---------------------------------------------------------------------------
def build_program(probe=False):
    nc = bass.Bass("TRN2", target_bir_lowering=False, debug=False, num_devices=NCORE)

    def inp(name, shape, d=dt.float32):
        return nc.declare_dram_parameter(name, list(shape), d, isOutput=False)

    bf = dt.bfloat16
    f8 = dt.float8e4
    xim_d = inp("xim", [27, B * 256], bf)          # host im2col of x
    w1_d = inp("w1col", [27, 64], bf)
    b1_d = inp("b1", [64, 1])
    w2_d = inp("w2col", [9 * 64, 128], bf)          # 9 taps stacked
    b2_d = inp("b2", [128, 1])
    fw_d = inp("feat_w", [128, 512], bf)            # pre-scaled by 1/64
    fb_d = inp("feat_b", [128, 4])
    wi_d = inp("wi", [128, 4 * 192], bf)
    wh_d = inp("wh", [64, 192], bf)
    bz_d = inp("bias_z", [64, 1])
    br2_d = inp("bias_r", [64, 1])
    bin_d = inp("bias_in", [64, 1])
    bhn_d = inp("bias_hn", [64, 1])
    wsrab_d = inp("wsrab", [65, 10], bf)
    wq_d = inp("wq", [128, 4 * 512], bf)
    wk_d = inp("wk", [128, 4 * 512], bf)
    wv_d = inp("wv", [128, 4 * 512], bf)
    wo_d = inp("wo", [128, 16 * 128], bf)           # [c-chunk, (k,m) blocks]
    bo_d = inp("bo", [128, 4])
    wr_d = inp("wr", [128, 16 * 128], bf)
    br_d = inp("br", [128, 4])
    wg_d = inp("wg", [128, 48 * 128], bf)           # [(k=12, m=4) blocks]
    bg_d = inp("bg", [128, 4])
    wc_d = inp("wc", [128, 48 * 128], bf)
    bc_d = inp("bc", [128, 4])
    wcls_d = inp("wcls", [128, 4 * 100], bf)
    bcls_d = inp("bcls", [100, 1])
    edge_d = inp("edge_tile", [128, 64], bf)        # (j, a) layout

    y_d = nc.declare_dram_parameter("y", [B, OUT], dt.float32, isOutput=True)
    if probe:
        pr_feats = nc.declare_dram_parameter("p_feats", [512, B], dt.float32, isOutput=True)
        pr_h = [nc.declare_dram_parameter(f"p_h{t}", [512, B], dt.float32, isOutput=True)
                for t in range(T)]
        pr_msg = nc.declare_dram_parameter("p_msg", [B, C], dt.float32, isOutput=True)
        pr_cs = nc.declare_dram_parameter("p_cs", [64, B], dt.float32, isOutput=True)

    with tile.TileContext(nc) as tc, \
         nc.allow_low_precision(reason="bf16 pipeline; reductions accumulate fp32 internally"):
        with tc.tile_pool(name="wp", bufs=1) as wp, \
             tc.tile_pool(name="dram", bufs=1, space="DRAM") as dram:

            # ---- early: identity, PE warm-up, skew-absorbing dummy AG ----
            ident16 = wp.tile([128, 128], bf);        make_identity(nc, ident16[:])
            ident32 = wp.tile([128, 128], dt.float32); make_identity(nc, ident32[:])
            warm_in = dram.tile([128, 8], bf, name="warm_in")
            warm_out = dram.tile([NCORE * 128, 8], bf, name="warm_out",
                                 addr_space="Shared")
            nc.gpsimd.collective_compute(
                "AllGather", ALU.bypass,
                replica_groups=[list(range(NCORE))],
                ins=[warm_in[:]], outs=[warm_out[:]])
            # ---------------- persistent weight/state tiles ----------------
            w1 = wp.tile([27, 64], bf);               nc.sync.dma_start(w1[:], w1_d[:])
            b1 = wp.tile([64, 1], dt.float32);        nc.sync.dma_start(b1[:], b1_d[:])
            w2t = []
            for tap in range(9):
                w2t.append(wp.tile([64, 128], bf, name=f"w2_{tap}"))
                nc.sync.dma_start(w2t[tap][:], w2_d[tap * 64:(tap + 1) * 64, :])
            b2 = wp.tile([128, 1], dt.float32);       nc.sync.dma_start(b2[:], b2_d[:])
            fw = wp.tile([128, 512], bf);             nc.gpsimd.dma_start(fw[:], fw_d[:])
            fb = wp.tile([128, 4], dt.float32);       nc.gpsimd.dma_start(fb[:], fb_d[:])
            wi = wp.tile([128, 4 * 192], bf);         nc.gpsimd.dma_start(wi[:], wi_d[:])
            wh = wp.tile([64, 192], bf);              nc.gpsimd.dma_start(wh[:], wh_d[:])
            bz_ = wp.tile([64, 1], dt.float32);       nc.gpsimd.dma_start(bz_[:], bz_d[:])
            br2 = wp.tile([64, 1], dt.float32);       nc.gpsimd.dma_start(br2[:], br2_d[:])
            bin_ = wp.tile([64, 1], dt.float32);      nc.gpsimd.dma_start(bin_[:], bin_d[:])
            bhn = wp.tile([64, 1], dt.float32);       nc.gpsimd.dma_start(bhn[:], bhn_d[:])
            wsrab = wp.tile([65, 10], bf);            nc.gpsimd.dma_start(wsrab[:], wsrab_d[:])
            wq = wp.tile([128, 2048], bf);            nc.gpsimd.dma_start(wq[:], wq_d[:])
            wk = wp.tile([128, 2048], bf);            nc.gpsimd.dma_start(wk[:], wk_d[:])
            wv = wp.tile([128, 2048], bf);            nc.gpsimd.dma_start(wv[:], wv_d[:])
            wo = wp.tile([128, 2048], bf);            nc.gpsimd.dma_start(wo[:], wo_d[:])
            bo = wp.tile([128, 4], dt.float32);       nc.gpsimd.dma_start(bo[:], bo_d[:])
            wr = wp.tile([128, 2048], bf);            nc.gpsimd.dma_start(wr[:], wr_d[:])
            br = wp.tile([128, 4], dt.float32);       nc.gpsimd.dma_start(br[:], br_d[:])
            wg = wp.tile([128, 6144], bf);            nc.gpsimd.dma_start(wg[:], wg_d[:])
            bg = wp.tile([128, 4], dt.float32);       nc.gpsimd.dma_start(bg[:], bg_d[:])
            wc = wp.tile([128, 6144], bf);            nc.gpsimd.dma_start(wc[:], wc_d[:])
            bc = wp.tile([128, 4], dt.float32);       nc.gpsimd.dma_start(bc[:], bc_d[:])
            wcls = wp.tile([128, 400], bf);           nc.gpsimd.dma_start(wcls[:], wcls_d[:])
            bcls = wp.tile([100, 1], dt.float32);     nc.gpsimd.dma_start(bcls[:], bcls_d[:])
            edge = wp.tile([128, 64], bf);            nc.gpsimd.dma_start(edge[:], edge_d[:])
            feats = [wp.tile([128, B], bf, name=f"feats{m}") for m in range(KM)]
            msum = [wp.tile([128, B], dt.float32, name=f"msum{m}") for m in range(KM)]
            cs16 = wp.tile([65, B], bf)
            nc.vector.memset(cs16[0:64, :], 0.0)
            nc.vector.memset(cs16[64:65, :], 1.0)
            pooled = wp.tile([128, B], bf)

            # ---------------- conv1 + conv2 + feats ----------------
            # h1x layout: [64c, (b, y(17), q(2), x'(9))] where input col u of
            # the conv2 input = (q=u%2, x'=u//2); y=16 row and (q0,x'=8) col
            # are the hi-padding (zero).
            with tc.tile_pool(name="cv", bufs=2) as cv, \
                 tc.tile_pool(name="cvh", bufs=2) as cvh, \
                 tc.tile_pool(name="cvs", bufs=2) as cvs, \
                 tc.tile_pool(name="pc1", bufs=3, space="PSUM") as pc1, \
                 tc.tile_pool(name="pc2", bufs=2, space="PSUM") as pc2:
                for g in range(NG):
                    if g == NG - 1:
                        # second skew-absorbing barrier: re-syncs cores while
                        # the last conv group still has plenty of work
                        warm2_in = dram.tile([128, 8], bf, name="warm2_in")
                        warm2_out = dram.tile([NCORE * 128, 8], bf,
                                              name="warm2_out", addr_space="Shared")
                        nc.gpsimd.collective_compute(
                            "AllGather", ALU.bypass,
                            replica_groups=[list(range(NCORE))],
                            ins=[warm2_in[:]], outs=[warm2_out[:]])
                    z = cv.tile([27, GB * 256], bf, tag="z")
                    nc.sync.dma_start(z[:], xim_d[:, g * GB * 256:(g + 1) * GB * 256])
                    h1x = cvh.tile([64, GB * 17 * 2 * 9], bf, tag="h1x")
                    h1v = h1x[:].rearrange("c (b y q x) -> c b y q x",
                                           b=GB, y=17, q=2, x=9)
                    # zero the hi-pad: y=16 plane and (q=0, x'=8) column
                    nc.vector.memset(h1v[:, :, 16, :, :], 0.0)
                    nc.vector.memset(h1v[:, :, 0:16, 0, 8:9], 0.0)
                    for i0 in range(0, GB, 8):
                        # conv1 for images i0..i0+8 (2 per matmul)
                        for i2 in range(i0, i0 + 8, 2):
                            ps = pc1.tile([64, 512], dt.float32, tag="pc1")
                            nc.tensor.matmul(ps[:], w1[:],
                                             z[:, i2 * 256:(i2 + 2) * 256],
                                             start=True, stop=True)
                            # relu + bias, written into parity-split fp8 layout
                            src = ps[:].rearrange("c (b r x q) -> c b r q x",
                                                  b=2, r=16, x=8, q=2)
                            dst = h1v[:, i2:i2 + 2, 0:16, :, 0:8]
                            if (i2 // 2) % 2 == 0:
                                nc.scalar.activation(dst, src, AF.Relu,
                                                     bias=b1[:, 0:1])
                            else:
                                nc.vector.tensor_scalar(
                                    out=dst, in0=src, scalar1=b1[:, 0:1],
                                    scalar2=0.0, op0=ALU.add, op1=ALU.max)
                        # conv2 for images i0..i0+8: 9-tap accumulation
                        ps2 = pc2.tile([128, 512], dt.float32, tag="pc2")
                        ps2v = ps2[:].rearrange("c (b y x) -> c b y x", b=8, y=8, x=8)
                        for tap in range(9):
                            dy, dx = tap // 3, tap % 3
                            q_t, x0 = dx % 2, dx // 2
                            rhs = h1v[:, i0:i0 + 8, dy:dy + 15:2, q_t, x0:x0 + 8]
                            nc.tensor.matmul(ps2v, w2t[tap][:], rhs,
                                             start=(tap == 0), stop=(tap == 8))
                        h2r = cvs.tile([128, 512], bf, tag="h2r")
                        nc.scalar.activation(h2r[:], ps2[:], AF.Relu, bias=b2[:, 0:1])
                        nc.vector.tensor_reduce(
                            out=pooled[:, g * GB + i0:g * GB + i0 + 8],
                            in_=h2r[:].rearrange("c (b s) -> c b s", b=8, s=64),
                            axis=AX.X, op=ALU.add)
                # feats = relu(fw.T @ pooled + fb)   (1/64 folded into fw)
                for m in range(KM):
                    psf = pc2.tile([128, 512], dt.float32, tag="pc2")
                    nc.tensor.matmul(psf[:, 0:B], fw[:, m * 128:(m + 1) * 128],
                                     pooled[:], start=True, stop=True)
                    nc.scalar.activation(feats[m][:], psf[:, 0:B], AF.Relu,
                                         bias=fb[:, m:m + 1])

            if probe:
                prtmp = wp.tile([128, B], dt.float32, name="prtmp")
                for m in range(KM):
                    nc.scalar.copy(prtmp[:], feats[m][:])
                    nc.sync.dma_start(pr_feats[m * 128:(m + 1) * 128, :], prtmp[:])

            # ---------------- recurrent steps ----------------
            h = feats  # h_0
            with tc.tile_pool(name="st", bufs=1) as st, \
                 tc.tile_pool(name="att", bufs=2) as att, \
                 tc.tile_pool(name="vjp", bufs=9) as vjp, \
                 tc.tile_pool(name="hp", bufs=2) as hp, \
                 tc.tile_pool(name="ps_b", bufs=2, space="PSUM") as ps_b, \
                 tc.tile_pool(name="ps_gc", bufs=4, space="PSUM") as ps_gc, \
                 tc.tile_pool(name="ps_tp", bufs=2, space="PSUM") as ps_tp:
                for t in range(T):
                    # ---- GRU controller (bf16 matmuls) ----
                    pz = ps_tp.tile([64, B], dt.float32, tag="tp", name=f"pz{t}")
                    for k in range(KM):
                        nc.tensor.matmul(pz[:], wi[:, k * 192:k * 192 + 64],
                                         h[k][:], start=(k == 0), stop=False)
                    nc.tensor.matmul(pz[:], wh[:, 0:64], cs16[0:64, :],
                                     start=False, stop=True)
                    zg = st.tile([64, B], dt.float32, tag="zg")
                    nc.scalar.activation(zg[:], pz[:], AF.Sigmoid, bias=bz_[:, 0:1])
                    pr_ = ps_tp.tile([64, B], dt.float32, tag="tp", name=f"pr{t}")
                    for k in range(KM):
                        nc.tensor.matmul(pr_[:], wi[:, k * 192 + 64:k * 192 + 128],
                                         h[k][:], start=(k == 0), stop=False)
                    nc.tensor.matmul(pr_[:], wh[:, 64:128], cs16[0:64, :],
                                     start=False, stop=True)
                    rg = st.tile([64, B], dt.float32, tag="rg")
                    nc.scalar.activation(rg[:], pr_[:], AF.Sigmoid, bias=br2[:, 0:1])
                    pin = ps_tp.tile([64, B], dt.float32, tag="tp", name=f"pin{t}")
                    for k in range(KM):
                        nc.tensor.matmul(pin[:], wi[:, k * 192 + 128:(k + 1) * 192],
                                         h[k][:], start=(k == 0), stop=(k == KM - 1))
                    inn = st.tile([64, B], dt.float32, tag="inn")
                    nc.scalar.activation(inn[:], pin[:], AF.Identity, bias=bin_[:, 0:1])
                    phn = ps_tp.tile([64, B], dt.float32, tag="tp", name=f"phn{t}")
                    nc.tensor.matmul(phn[:], wh[:, 128:192], cs16[0:64, :],
                                     start=True, stop=True)
                    hn = st.tile([64, B], dt.float32, tag="hn")
                    nc.scalar.activation(hn[:], phn[:], AF.Identity, bias=bhn[:, 0:1])
                    # n = tanh(inn + r*hn); cs += z*(n-cs)
                    ngate = st.tile([64, B], dt.float32, tag="ngate")
                    nc.vector.tensor_tensor(out=ngate[:], in0=rg[:], in1=hn[:],
                                            op=ALU.mult)
                    nc.vector.tensor_tensor(out=ngate[:], in0=ngate[:], in1=inn[:],
                                            op=ALU.add)
                    nc.scalar.activation(ngate[:], ngate[:], AF.Tanh)
                    tmp = st.tile([64, B], dt.float32, tag="tmpg")
                    nc.vector.tensor_tensor(out=tmp[:], in0=ngate[:],
                                            in1=cs16[0:64, :], op=ALU.subtract)
                    nc.vector.tensor_tensor(out=tmp[:], in0=tmp[:], in1=zg[:],
                                            op=ALU.mult)
                    nc.vector.tensor_tensor(out=cs16[0:64, :], in0=cs16[0:64, :],
                                            in1=tmp[:], op=ALU.add)
                    if probe and t == 0:
                        prcs = st.tile([64, B], dt.float32, tag="prcs")
                        nc.scalar.copy(prcs[:], cs16[0:64, :])
                        nc.sync.dma_start(pr_cs[:], prcs[:])

                    # ---- srab = (send|recv|ab) batch-major via bias-row trick ----
                    sr = [None] * NBCH      # fp32 [128, 2]: send, recv
                    ab16 = [None] * NBCH    # bf16 [128, 8]
                    ea16 = [None] * NBCH    # bf16 [128, 64] (j, a)
                    for ch in range(NBCH):
                        psr = ps_tp.tile([128, 16], dt.float32, tag="tp",
                                         name=f"psr{ch}_{t}")
                        nc.tensor.matmul(psr[:, 0:10], cs16[:, ch * 128:(ch + 1) * 128],
                                         wsrab[:], start=True, stop=True)
                        srt = st.tile([128, 2], dt.float32, tag=f"sr{ch}")
                        nc.scalar.activation(srt[:], psr[:, 0:2], AF.Sigmoid)
                        sr[ch] = srt
                        abt = st.tile([128, 8], bf, tag=f"ab{ch}")
                        nc.scalar.copy(abt[:], psr[:, 2:10])
                        ab16[ch] = abt

                    # ---- k,v (+send scaling) then per-chunk AllGather (fp8) ----
                    f8 = dt.float8e4
                    exout = []
                    kvx_c = []
                    for ch in range(NBCH):
                        cols = slice(ch * 128, (ch + 1) * 128)
                        kvx = att.tile([128, 1024], f8, tag="kvx", name=f"kvx{ch}_{t}")
                        pk = ps_b.tile([128, 512], dt.float32, tag="mm")
                        for k in range(KM):
                            nc.tensor.matmul(pk[:], h[k][:, cols],
                                             wk[:, k * 512:(k + 1) * 512],
                                             start=(k == 0), stop=(k == KM - 1))
                        # wk/wv are pre-scaled x64 into fp8 normal range
                        nc.scalar.copy(kvx[:, 0:512], pk[:])
                        exin = dram.tile([128, 1024], f8, name=f"exin{t}_{ch}")
                        exo = dram.tile([NCORE * 128, 1024], f8,
                                        name=f"exout{t}_{ch}", addr_space="Shared")
                        nc.sync.dma_start(exin[:, 0:512], kvx[:, 0:512])
                        pv = ps_b.tile([128, 512], dt.float32, tag="mm")
                        for k in range(KM):
                            nc.tensor.matmul(pv[:], h[k][:, cols],
                                             wv[:, k * 512:(k + 1) * 512],
                                             start=(k == 0), stop=(k == KM - 1))
                        nc.scalar.activation(kvx[:, 512:1024], pv[:], AF.Copy,
                                             scale=sr[ch][:, 0:1])
                        nc.sync.dma_start(exin[:, 512:1024], kvx[:, 512:1024])
                        nc.gpsimd.collective_compute(
                            "AllGather", ALU.bypass,
                            replica_groups=[list(range(NCORE))],
                            ins=[exin[:]], outs=[exo[:]])
                        exout.append(exo)
                        kvx_c.append(kvx)

                    # ---- ea = edge + ab, computed under the collectives ----
                    for ch in range(NBCH):
                        eat = st.tile([128, 64], bf, tag=f"ea{ch}",
                                      name=f"ea{ch}_{t}")
                        nc.vector.tensor_tensor(
                            out=eat[:].rearrange("p (j a) -> p j a", j=8),
                            in0=ab16[ch][:].unsqueeze(1).broadcast_to([128, 8, 8]),
                            in1=edge[:].rearrange("p (j a) -> p j a", j=8),
                            op=ALU.add)
                        ea16[ch] = eat

                    # ---- q (overlaps the collectives) ----
                    qc = [None] * NBCH
                    for ch in range(NBCH):
                        cols = slice(ch * 128, (ch + 1) * 128)
                        pq = ps_b.tile([128, 512], dt.float32, tag="mm")
                        for k in range(KM):
                            nc.tensor.matmul(pq[:], h[k][:, cols],
                                             wq[:, k * 512:(k + 1) * 512],
                                             start=(k == 0), stop=(k == KM - 1))
                        q = att.tile([128, 512], bf, tag="q", name=f"q{ch}_{t}")
                        nc.scalar.copy(q[:], pq[:])
                        qc[ch] = q

                    # ---- wg/wc partial accumulation over h and feats chunks
                    # (overlaps the collectives; finished after readout) ----
                    pgc = []
                    for m in range(KM):
                        pg = ps_gc.tile([128, 512], dt.float32, tag="gc",
                                        name=f"pgc{m}_{t}")
                        for k in range(8):
                            src = h[k] if k < 4 else feats[k - 4]
                            nc.tensor.matmul(
                                pg[:, 0:B],
                                wg[:, (k * 4 + m) * 128:(k * 4 + m + 1) * 128],
                                src[:], start=(k == 0), stop=False,
                                skip_group_check=True)
                        for k in range(8):
                            src = h[k] if k < 4 else feats[k - 4]
                            nc.tensor.matmul(
                                pg[:, B:2 * B],
                                wc[:, (k * 4 + m) * 128:(k * 4 + m + 1) * 128],
                                src[:], start=(k == 0), stop=False,
                                skip_group_check=True)
                        pgc.append(pg)

                    # ---- attention per chunk (all DVE, bf16) ----
                    msgf = [st.tile([128, B], bf, tag=f"msgf{m}",
                                    name=f"msgf{m}_{t}") for m in range(KM)]
                    for ch in range(NBCH):
                        q = qc[ch]
                        kvj = []
                        for j in range(NCORE):
                            rows = slice(j * 128, (j + 1) * 128)
                            kv = vjp.tile([128, 1024], dt.float8e4, tag="kvj",
                                          name=f"kv{j}_{ch}_{t}")
                            nc.sync.dma_start(kv[:], exout[ch][rows, :])
                            kvj.append(kv)
                        S16 = st.tile([128, 64], bf, tag=f"S{ch}")
                        for j in range(NCORE):
                            eng = nc.gpsimd if j == 7 else nc.vector
                            prod = att.tile([128, 512], bf, tag="prod")
                            eng.tensor_tensor(out=prod[:], in0=q[:],
                                              in1=kvj[j][:, 0:512],
                                              op=ALU.mult)
                            nc.vector.tensor_reduce(
                                out=S16[:, j * 8:(j + 1) * 8],
                                in_=prod[:].rearrange("p (a d) -> p a d", d=64),
                                axis=AX.X, op=ALU.add)
                        # S2 = 0.125*S + (edge + ab)
                        S2 = st.tile([128, 64], bf, tag=f"S2{ch}")
                        nc.vector.scalar_tensor_tensor(
                            out=S2[:], in0=S16[:], scalar=0.125 / 64.0,
                            in1=ea16[ch][:], op0=ALU.mult, op1=ALU.add)
                        Se = st.tile([128, 64], bf, tag=f"Se{ch}")
                        nc.scalar.activation(Se[:], S2[:], AF.Exp)
                        zt = st.tile([128, 8], dt.float32, tag=f"zt{ch}")
                        nc.vector.tensor_reduce(
                            out=zt[:],
                            in_=Se[:].rearrange("p (j a) -> p a j", j=8),
                            axis=AX.X, op=ALU.add)
                        nc.vector.reciprocal(zt[:], zt[:])
                        nc.vector.tensor_scalar(out=zt[:], in0=zt[:],
                                                scalar1=sr[ch][:, 1:2],
                                                scalar2=1.0 / 64.0,
                                                op0=ALU.mult, op1=ALU.mult)
                        # weighted sum of v_j with exp-scores, then normalize
                        pva = att.tile([128, 4096], bf, tag="pva")
                        for j in range(NCORE):
                            eng = nc.gpsimd if j >= 6 else nc.vector
                            eng.tensor_tensor(
                                out=pva[:, j * 512:(j + 1) * 512].rearrange(
                                    "p (a d) -> p a d", d=64),
                                in0=kvj[j][:, 512:1024].rearrange(
                                    "p (a d) -> p a d", d=64),
                                in1=Se[:, j * 8:(j + 1) * 8].unsqueeze(2)
                                    .broadcast_to([128, 8, 64]),
                                op=ALU.mult)
                        nc.vector.tensor_tensor(out=pva[:, 0:2048],
                                                in0=pva[:, 0:2048],
                                                in1=pva[:, 2048:4096], op=ALU.add)
                        nc.vector.tensor_tensor(out=pva[:, 0:1024],
                                                in0=pva[:, 0:1024],
                                                in1=pva[:, 1024:2048], op=ALU.add)
                        nc.vector.tensor_tensor(out=pva[:, 0:512],
                                                in0=pva[:, 0:512],
                                                in1=pva[:, 512:1024], op=ALU.add)
                        msg = att.tile([128, 512], bf, tag="msg", name=f"msg{ch}_{t}")
                        nc.vector.tensor_tensor(
                            out=msg[:].rearrange("p (a d) -> p a d", d=64),
                            in0=pva[:, 0:512].rearrange("p (a d) -> p a d", d=64),
                            in1=zt[:].unsqueeze(2).broadcast_to([128, 8, 64]),
                            op=ALU.mult)
                        if probe and t == 0:
                            prm = st.tile([128, 512], dt.float32, tag="prm")
                            nc.scalar.copy(prm[:], msg[:])
                            nc.sync.dma_start(pr_msg[ch * 128:(ch + 1) * 128, :],
                                              prm[:])
                        # transpose msg to feature-major for this chunk
                        for m in range(KM):
                            ptp = ps_tp.tile([128, 128], bf, tag="tp",
                                             name=f"ptp{m}_{ch}_{t}")
                            nc.tensor.transpose(
                                ptp[:], msg[:, m * 128:(m + 1) * 128], ident16[:])
                            nc.scalar.copy(msgf[m][:, ch * 128:(ch + 1) * 128],
                                           ptp[:])

                    # ---- wo + FIFO running mean + readout (per chunk so the
                    # ch0 half overlaps ch1's attention) ----
                    ms8 = [st.tile([128, B], bf, tag=f"ms8{m}", name=f"ms8{m}_{t}")
                           for m in range(KM)]
                    ro = [st.tile([128, B], bf, tag=f"ro{m}", name=f"ro{m}_{t}")
                          for m in range(KM)]
                    for ch in range(NBCH):
                        cols = slice(ch * 128, (ch + 1) * 128)
                        for m in range(KM):
                            pso = ps_b.tile([128, 512], dt.float32, tag="mm")
                            for k in range(KM):
                                nc.tensor.matmul(
                                    pso[:, 0:128],
                                    wo[:, (k * 4 + m) * 128:(k * 4 + m + 1) * 128],
                                    msgf[k][:, cols],
                                    start=(k == 0), stop=(k == KM - 1))
                            if t == 0:
                                nc.scalar.activation(msum[m][:, cols],
                                                     pso[:, 0:128],
                                                     AF.Identity,
                                                     bias=bo[:, m:m + 1])
                            else:
                                wot = st.tile([128, 128], dt.float32, tag="wot")
                                nc.scalar.activation(wot[:], pso[:, 0:128],
                                                     AF.Identity,
                                                     bias=bo[:, m:m + 1])
                                nc.vector.tensor_tensor(out=msum[m][:, cols],
                                                        in0=msum[m][:, cols],
                                                        in1=wot[:], op=ALU.add)
                            nc.scalar.mul(ms8[m][:, cols], msum[m][:, cols], 0.125)
                        for m in range(KM):
                            psr_ = ps_b.tile([128, 512], dt.float32, tag="mm")
                            for k in range(KM):
                                nc.tensor.matmul(
                                    psr_[:, 0:128],
                                    wr[:, (k * 4 + m) * 128:(k * 4 + m + 1) * 128],
                                    ms8[k][:, cols],
                                    start=(k == 0), stop=(k == KM - 1))
                            nc.scalar.activation(ro[m][:, cols], psr_[:, 0:128],
                                                 AF.Identity, bias=br[:, m:m + 1])

                    # ---- finish wg/wc with readout chunks; gated update ----
                    hnew = [hp.tile([128, B], bf, tag=f"h{m}", name=f"h{m}_{t}")
                            for m in range(KM)]
                    for m in range(KM):
                        pg = pgc[m]
                        for k in range(8, 12):
                            nc.tensor.matmul(
                                pg[:, 0:B],
                                wg[:, (k * 4 + m) * 128:(k * 4 + m + 1) * 128],
                                ro[k - 8][:], start=False, stop=(k == 11),
                                skip_group_check=True)
                        for k in range(8, 12):
                            nc.tensor.matmul(
                                pg[:, B:2 * B],
                                wc[:, (k * 4 + m) * 128:(k * 4 + m + 1) * 128],
                                ro[k - 8][:], start=False, stop=(k == 11),
                                skip_group_check=True)
                        gt = st.tile([128, B], dt.float32, tag=f"g{m}")
                        nc.scalar.activation(gt[:], pg[:, 0:B], AF.Sigmoid,
                                             bias=bg[:, m:m + 1])
                        cand = st.tile([128, B], dt.float32, tag="cand")
                        nc.scalar.activation(cand[:], pg[:, B:2 * B], AF.Tanh,
                                             bias=bc[:, m:m + 1])
                        nc.vector.tensor_tensor(out=cand[:], in0=cand[:], in1=h[m][:],
                                                op=ALU.subtract)
                        nc.vector.tensor_tensor(out=cand[:], in0=cand[:], in1=gt[:],
                                                op=ALU.mult)
                        nc.vector.tensor_tensor(out=hnew[m][:], in0=h[m][:],
                                                in1=cand[:], op=ALU.add)
                    h = hnew
                    if probe:
                        prh = st.tile([128, B], dt.float32, tag="prh")
                        for m in range(KM):
                            nc.scalar.copy(prh[:], h[m][:])
                            nc.sync.dma_start(pr_h[t][m * 128:(m + 1) * 128, :],
                                              prh[:])

                # ---- classifier ----
                pcl = ps_b.tile([128, 512], dt.float32, tag="mm", name="pcl")
                for k in range(KM):
                    nc.tensor.matmul(pcl[0:100, 0:B], wcls[:, k * 100:(k + 1) * 100],
                                     h[k][:], start=(k == 0), stop=(k == KM - 1))
                lg = st.tile([100, B], dt.float32, tag="lg")
                nc.scalar.activation(lg[:], pcl[0:100, 0:B], AF.Identity,
                                     bias=bcls[:, 0:1])
                for ch in range(NBCH):
                    ptp = ps_tp.tile([128, 128], dt.float32, tag="tp",
                                     name=f"ptpcls{ch}")
                    nc.tensor.transpose(ptp[:], lg[:, ch * 128:(ch + 1) * 128],
                                        ident32[0:100, :])
                    lgb = st.tile([128, 100], dt.float32, tag="lgb")
                    nc.scalar.copy(lgb[:], ptp[:, 0:100])
                    nc.sync.dma_start(y_d[ch * 128:(ch + 1) * 128, :], lgb[:])

    _split_multiwaits(nc)
    return nc


# ---------------------------------------------------------------------------
# Host-side input preparation (pure layout: slice/reshape/transpose/concat)
# ---------------------------------------------------------------------------
def prep_core_inputs(inputs, n):
    f32 = np.float32
    bf16 = ml_dtypes.bfloat16
    g = lambda k: np.ascontiguousarray(np.asarray(inputs[k], f32))
    x = g("x")  # [B, 3, 32, 32]
    xpad = np.zeros((B, 3, 33, 33), f32)
    xpad[:, :, 0:32, 0:32] = x
    xim = np.empty((27, B * 256), f32)
    for dy in range(3):
        for dx in range(3):
            blk = xpad[:, :, dy:dy + 31:2, dx:dx + 31:2]  # [B,3,16,16]
            for ci in range(3):
                xim[ci * 9 + dy * 3 + dx] = blk[:, ci].reshape(B * 256)
    w1 = g("conv1_w")[n]          # [64,3,3,3]
    w1col = np.ascontiguousarray(w1.transpose(1, 2, 3, 0).reshape(27, 64))
    w2 = g("conv2_w")[n]          # [128,64,3,3]
    w2col = np.ascontiguousarray(
        np.concatenate([w2[:, :, tap // 3, tap % 3].T for tap in range(9)], 0))
    wi = g("ctrl_wi")[n]
    wh = g("ctrl_wh")[n]
    bi = g("ctrl_bi")[n]
    bh = g("ctrl_bh")[n]
    wsrab = np.zeros((65, 10), f32)
    wsrab[0:64, 0:1] = g("send_w")[n]
    wsrab[0:64, 1:2] = g("recv_w")[n]
    wsrab[0:64, 2:10] = g("abias_w")[n]
    wsrab[64, 0] = g("send_b")[n][0]
    wsrab[64, 1] = g("recv_b")[n][0]
    wsrab[64, 2:10] = g("abias_b")[n]
    edge_row = g("edge_logits")[n]           # edge_logits[i=n, j]
    # (j, a) layout: index = j*NH + a
    edge_tile = np.ascontiguousarray(
        np.tile(np.repeat(edge_row, NH)[None, :], (128, 1)).astype(f32))

    def pack_k(w, kchunks, ncols):  # [K, ncols] -> [128, kchunks*ncols]
        return np.ascontiguousarray(
            np.concatenate([w[k * 128:(k + 1) * 128] for k in range(kchunks)], 1))

    def pack_blocks(w, kchunks):
        # [K, 512] -> [128, kchunks*4*128]: block (k, m) at cols (k*4+m)*128
        blks = []
        for k in range(kchunks):
            for m in range(KM):
                blks.append(w[k * 128:(k + 1) * 128, m * 128:(m + 1) * 128])
        return np.ascontiguousarray(np.concatenate(blks, 1))

    def pack_b(b):
        return np.ascontiguousarray(b.reshape(4, 128).T)

    return {
        "xim": xim.astype(bf16),
        "w1col": w1col.astype(bf16),
        "b1": g("conv1_b")[n].reshape(64, 1),
        "w2col": w2col.astype(bf16),
        "b2": g("conv2_b")[n].reshape(128, 1),
        "feat_w": (g("feat_w")[n] / 64.0).astype(bf16),
        "feat_b": pack_b(g("feat_b")[n]),
        "wi": pack_k(wi, 4, 192).astype(bf16),
        "wh": wh.astype(bf16),
        "bias_z": (bi[0:64] + bh[0:64]).reshape(64, 1),
        "bias_r": (bi[64:128] + bh[64:128]).reshape(64, 1),
        "bias_in": bi[128:192].reshape(64, 1),
        "bias_hn": bh[128:192].reshape(64, 1),
        "wsrab": wsrab.astype(bf16),
        "wq": pack_k(g("wq")[n], 4, 512).astype(bf16),
        "wk": (pack_k(g("wk")[n], 4, 512) * 64.0).astype(bf16),
        "wv": (pack_k(g("wv")[n], 4, 512) * 64.0).astype(bf16),
        "wo": pack_blocks(g("wo")[n], 4).astype(bf16),
        "bo": pack_b(g("bo")[n]),
        "wr": pack_blocks(g("wr")[n], 4).astype(bf16),
        "br": pack_b(g("br")[n]),
        "wg": pack_blocks(g("wg")[n], 12).astype(bf16),
        "bg": pack_b(g("bg")[n]),
        "wc": pack_blocks(g("wc")[n], 12).astype(bf16),
        "bc": pack_b(g("bc")[n]),
        "wcls": pack_k(g("wcls")[n], 4, 100).astype(bf16),
        "bcls": g("bcls")[n].reshape(100, 1),
        "edge_tile": edge_tile.astype(bf16),
    }


def kernel(**inputs):
    inputs.pop("step", None)
    probe = bool(int(os.environ.get("KERNEL_PROBE", "0")))
    key = ("prog", probe)
    if key not in _CACHE:
        _CACHE[key] = build_program(probe=probe)
    nc = _CACHE[key]
    in_maps = [prep_core_inputs(inputs, n) for n in range(NCORE)]
    res = run_bass_kernel_spmd(nc, in_maps, list(range(NCORE)), trace=TRACE)
    kernel.last_results = res
    out = np.stack([res.results[n]["y"] for n in range(NCORE)], 0)
    return out.astype(np.float32)
